# revision 1
# baseline (speedup 1.0000x reference)
"""Trainium2 Bass kernel for nn_ContrastiveUnlearnLoss.

Reference math (B=8192, D=512):
    sim = l2norm(h_f) @ l2norm(h_r).T                     # [B, B]
    p_msk = labels_f[:,None] == labels_r[None,:]
    e = exp(sim); sum_p = sum(where(p_msk, e, 0), axis=1)
    log_terms = log(e / sum_p[:,None] + EPS)
    loss_rows = -sum(where(~p_msk, log_terms, 0), axis=1) / (n_count + 1)
    return loss_rows[-1] / B          # <-- ONLY the last row survives

So the output is a scalar depending only on u = h_f[-1], c = labels_f[-1],
and all of h_r / labels_r.  With S = sum_p[-1] (global masked sum) and
sim_j = cos(u, h_r[j]):

    log(e_j/S + EPS) = log(e_j + EPS*S) - log(S)
                     = sim_j + log1p(EPS*S*exp(-sim_j)) - log(S)
                     = sim_j + EPS*S*exp(-sim_j) - log(S)   (+O(1e-12))

    sum_neg log_terms = A + EPS*S*B - n*log(S)
      with  A = sum_neg sim_j,  B = sum_neg exp(-sim_j),  n = #neg

Sharding: h_r rows split 8 ways (1024 rows/core, 2MB/core, memory-bound).
Each core computes the 4 partial sums [P, A, B, n] over its shard on
device; the host all-reduces the 4 scalars and forms the loss.
"""

import numpy as np

import concourse.bass as bass
import concourse.mybir as mybir
from concourse.tile import TileContext
from concourse.bass_utils import run_bass_kernel_spmd

B_TOTAL = 8192
D = 512
N_CORES = 8
ROWS_PER_CORE = B_TOTAL // N_CORES          # 1024
ROW_TILES = ROWS_PER_CORE // 128            # 8 tiles of [128, 512]
EPS = 1e-9
COS_EPS = 1e-8

F32 = mybir.dt.float32
AF = mybir.ActivationFunctionType
ALU = mybir.AluOpType

_MW_CTR = [0]


def _split_multiwaits(nc):
    """This container's walrus accepts at most ONE sync wait per
    instruction ("Too many sync wait commands"), but Tile's tail Drain
    waits on every DMA-queue semaphore.  Hoist all-but-the-last wait onto
    single-wait NoOps on the same engine queue, placed just before."""
    fn = nc.m.functions[0]
    for blk in fn.blocks:
        out = []
        changed = False
        for inst in blk.instructions:
            si = inst.sync_info
            waits = list(si.on_wait) if (si is not None and si.on_wait) else []
            if len(waits) > 1:
                changed = True
                for w in waits[:-1]:
                    _MW_CTR[0] += 1
                    nop = mybir.InstNoOp(
                        name=f"mwsplit-{_MW_CTR[0]}", ins=[], outs=[]
                    )
                    nop.engine = inst.engine
                    nop.sync_info = mybir.SyncInfo(on_wait=[w], on_update=[])
                    out.append(nop)
                si.on_wait = [waits[-1]]
            out.append(inst)
        if changed:
            blk.instructions = out
    return nc


def _build_nc(label_last: float, walrus_fix: bool = True) -> bass.Bass:
    """Per-core program: hr shard [1024,512] + broadcast u [128,512] +
    labels layout [128,8] -> out4 [1,4] = [P, A, B, n] partial sums."""
    nc = bass.Bass(trn_type="TRN2")

    hr = nc.dram_tensor("hr", [ROWS_PER_CORE, D], F32, kind="ExternalInput")
    un = nc.dram_tensor("un", [1, D], F32, kind="ExternalInput")
    lab = nc.dram_tensor("lab", [128, ROW_TILES], F32, kind="ExternalInput")
    out4 = nc.dram_tensor("out4", [1, 4], F32, kind="ExternalOutput")

    with TileContext(nc) as tc:
        with (
            tc.tile_pool(name="const", bufs=1) as const,
            tc.tile_pool(name="x", bufs=4) as xpool,
            tc.tile_pool(name="scratch", bufs=2) as spool,
            tc.tile_pool(name="small", bufs=1) as small,
            tc.tile_pool(name="psum", bufs=1, space="PSUM") as ppool,
        ):
            # broadcast u_n [1,512] -> [128,512] on-device: ones-matmul on
            # the (otherwise idle) PE, then one copy into SBUF.  Saves the
            # 256KB broadcast DMA.
            un_row = const.tile([1, D], F32)
            nc.sync.dma_start(un_row[:], un.ap())
            lab_t = const.tile([128, ROW_TILES], F32)
            nc.sync.dma_start(lab_t[:], lab.ap())
            ones_row = const.tile([1, 128], F32)
            nc.any.memset(ones_row[:], 1.0)
            ps_un = ppool.tile([128, D], F32, tag="psun")
            nc.tensor.matmul(ps_un[:, :], ones_row[:], un_row[:])
            un_t = const.tile([128, D], F32)
            nc.scalar.copy(un_t[:], ps_un[:, :])

            ssq = small.tile([128, ROW_TILES], F32)   # row sum-of-squares
            dot = small.tile([128, ROW_TILES], F32)   # row dot with u_n

            # masks depend only on labels -> compute during the stream
            pm = small.tile([128, ROW_TILES], F32)
            nc.vector.tensor_scalar(
                out=pm[:], in0=lab_t[:], scalar1=float(label_last),
                scalar2=None, op0=ALU.is_equal
            )
            nm = small.tile([128, ROW_TILES], F32)
            nc.vector.tensor_scalar(
                out=nm[:], in0=pm[:], scalar1=-1.0, scalar2=1.0,
                op0=ALU.mult, op1=ALU.add
            )

            # hr stream: HWDGE descriptor-gen costs ~625ns per dma_start on
            # one shared generator, so batch 2 row-tiles per DMA (4 DMAs,
            # 512KB each) to keep generation off the critical path while
            # retaining fine-enough completion granularity.
            GRP = 2
            hr_r = hr.rearrange("(a p) d -> p a d", p=128)  # [128, 8, 512]
            for g in range(ROW_TILES // GRP):
                xg = xpool.tile([128, GRP * D], F32, tag="x")
                nc.sync.dma_start(xg[:], hr_r[:, g * GRP:(g + 1) * GRP, :])
                for i in range(GRP):
                    t = g * GRP + i
                    x = xg[:, i * D:(i + 1) * D]
                    # sum(x^2) along free dim on the scalar engine
                    sq = spool.tile([128, D], F32, tag="sq")
                    nc.scalar.activation(
                        sq[:], x, AF.Square, accum_out=ssq[:, t:t + 1]
                    )
                    # dot(x, u_n) along free dim on the vector engine (fused)
                    mo = spool.tile([128, D], F32, tag="mo")
                    nc.vector.scalar_tensor_tensor(
                        out=mo[:], in0=x, scalar=1.0, in1=un_t[:],
                        op0=ALU.mult, op1=ALU.mult,
                        accum_out=dot[:, t:t + 1],
                    )

            # sim = dot / sqrt(ssq).  The reference clamps the norm at 1e-8;
            # ||h_r[j]|| ~ sqrt(512) >> 1e-8 for this distribution, and
            # sqrt(ssq) > 0 exactly unless the row is all-zero, so the clamp
            # is numerically dead here.  (u_n is normalized on host with the
            # exact clamped formula.)
            rs = small.tile([128, ROW_TILES], F32)
            nc.scalar.activation(rs[:], ssq[:], AF.Sqrt)
            rinv = small.tile([128, ROW_TILES], F32)
            nc.vector.reciprocal(rinv[:], rs[:])
            sim = small.tile([128, ROW_TILES], F32)
            nc.vector.tensor_mul(sim[:], dot[:], rinv[:])

            e = small.tile([128, ROW_TILES], F32)
            nc.scalar.activation(e[:], sim[:], AF.Exp)
            em = small.tile([128, ROW_TILES], F32)
            nc.scalar.activation(em[:], sim[:], AF.Exp, scale=-1.0)

            # per-partition partial sums -> par4 columns [P, A, B, n]
            par4 = small.tile([128, 4], F32)
            t0 = spool.tile([128, ROW_TILES], F32, tag="tmp")
            nc.vector.scalar_tensor_tensor(
                out=t0[:], in0=pm[:], scalar=1.0, in1=e[:],
                op0=ALU.mult, op1=ALU.mult, accum_out=par4[:, 0:1],
            )
            t1 = spool.tile([128, ROW_TILES], F32, tag="tmp")
            nc.vector.scalar_tensor_tensor(
                out=t1[:], in0=nm[:], scalar=1.0, in1=sim[:],
                op0=ALU.mult, op1=ALU.mult, accum_out=par4[:, 1:2],
            )
            t2 = spool.tile([128, ROW_TILES], F32, tag="tmp")
            nc.vector.scalar_tensor_tensor(
                out=t2[:], in0=nm[:], scalar=1.0, in1=em[:],
                op0=ALU.mult, op1=ALU.mult, accum_out=par4[:, 2:3],
            )
            nc.vector.tensor_reduce(
                par4[:, 3:4], nm[:], axis=mybir.AxisListType.X, op=ALU.add
            )

            # reduce across partitions with ones-matmul: out[1,4]
            ones = small.tile([128, 1], F32)
            nc.any.memset(ones[:], 1.0)
            ps = ppool.tile([128, 4], F32)
            nc.tensor.matmul(ps[:1, :], ones[:], par4[:])
            outt = small.tile([1, 4], F32)
            nc.any.tensor_copy(outt[:], ps[:1, :])
            nc.sync.dma_start(out4.ap(), outt[:])

    if walrus_fix:
        _split_multiwaits(nc)
    return nc


def _build_nc_raw(label_last: float, walrus_fix: bool = True) -> bass.Bass:
    """Hand-scheduled (no TileContext) per-core program.  Avoids Tile's
    kernel-tail drain + EVSEM barrier and preamble; pipelines hr DMA groups
    against ACT (ssq) and DVE (dots) streams; masked sums come out of two
    ACT accum-activations via mask folding; the 128-partition reduction of
    the 4 partials happens on the host during the cross-core all-reduce.
    """
    from contextlib import ExitStack

    nc = bass.Bass(trn_type="TRN2")

    hr = nc.dram_tensor("hr", [ROWS_PER_CORE, D], F32, kind="ExternalInput")
    un = nc.dram_tensor("un", [128, D], F32, kind="ExternalInput")
    lab = nc.dram_tensor("lab", [128, ROW_TILES], F32, kind="ExternalInput")
    out4 = nc.dram_tensor("out4", [128, 4], F32, kind="ExternalOutput")
    hr_r = hr.rearrange("(a p) d -> p a d", p=128)   # [128, 8, 512]

    # hr DMA groups (tiles): single-tile DMAs keep the BW train packed and
    # completion granularity fine; HWDGE gen (625ns each) stays just ahead
    # of the 728ns transfers.
    GROUPS = [(t, t + 1) for t in range(ROW_TILES)]

    # Masked sums via input folding: simp = sim - 40*nm pushes negatives to
    # ~-40, so  P = sum_pos e^sim   = accum(exp(simp))          (exact for
    # positives; e^-40 ~ 4e-18 is invisible next to e^sim in f32), and
    #     B = sum_neg e^-sim = accum(exp(-simp - 40))   (positives get
    # e^(-sim-40) ~ 0; negatives e^(-sim+40-40) = e^-sim up to one f32
    # rounding of (sim-40)+40, a ~2e-6 absolute exponent error on a term
    # that only enters the loss scaled by EPS*S).
    MASK_BIG = 40.0
    # Abs_reciprocal_sqrt would fuse sqrt+reciprocal into one ACT op
    # (-263ns modeled) but CoreSim can't simulate it and the ACT-table
    # accuracy is unvalidated; keep the exact sqrt + DVE reciprocal.
    RSQRT_ON_ACT = False

    with ExitStack() as ctx:
        e = ctx.enter_context
        xbuf = e(nc.sbuf_tensor([128, ROW_TILES * D], F32))
        un_t = e(nc.sbuf_tensor([128, D], F32))
        lab_t = e(nc.sbuf_tensor([128, ROW_TILES], F32))
        # per-op dummy outs (race detector rejects same-engine WAW reuse)
        sq = e(nc.sbuf_tensor([128, ROW_TILES * D], F32))
        mo = e(nc.sbuf_tensor([128, ROW_TILES * D], F32))
        ssq = e(nc.sbuf_tensor([128, ROW_TILES], F32))
        dot = e(nc.sbuf_tensor([128, ROW_TILES], F32))
        rs = e(nc.sbuf_tensor([128, ROW_TILES], F32))
        rinv = e(nc.sbuf_tensor([128, ROW_TILES], F32))
        sim = e(nc.sbuf_tensor([128, ROW_TILES], F32))
        nm = e(nc.sbuf_tensor([128, ROW_TILES], F32))
        simp = e(nc.sbuf_tensor([128, ROW_TILES], F32))
        ev = e(nc.sbuf_tensor([128, ROW_TILES], F32))
        em = e(nc.sbuf_tensor([128, ROW_TILES], F32))
        tt1 = e(nc.sbuf_tensor([128, ROW_TILES], F32))
        par4 = e(nc.sbuf_tensor([128, 4], F32))
        zeros = e(nc.sbuf_tensor([128, 1], F32))
        neg40 = e(nc.sbuf_tensor([128, 1], F32))

        s_hr = [e(nc.semaphore(name=f"s_hr{g}")) for g in range(len(GROUPS))]
        s_un = e(nc.semaphore(name="s_un"))
        s_lab = e(nc.semaphore(name="s_lab"))
        s_const = e(nc.semaphore(name="s_const"))
        s_sact = e(nc.semaphore(name="s_sact"))
        s_mask = e(nc.semaphore(name="s_mask"))
        s_rv = e(nc.semaphore(name="s_rv"))
        s_rs = e(nc.semaphore(name="s_rs"))
        s_sim = e(nc.semaphore(name="s_sim"))
        s_fold = e(nc.semaphore(name="s_fold"))
        s_par4 = e(nc.semaphore(name="s_par4"))
        s_out = e(nc.semaphore(name="s_out"))

        def grp_of(t):
            for g, (a, b) in enumerate(GROUPS):
                if a <= t < b:
                    return g
            raise AssertionError(t)

        with nc.Block() as block:

            @block.sync
            def _(sync):
                # hr tiles 0-2 first (feed ACT asap), u_n broadcast after
                # (DVE dots compress behind it), labels last (only needed
                # by the late mask ops).
                def hr_dma(g):
                    a, b = GROUPS[g]
                    sync.dma_start(
                        xbuf[:, a * D:b * D], hr_r[:, a:b, :]
                    ).then_inc(s_hr[g], 16)

                for g in (0, 1, 2):
                    hr_dma(g)
                sync.dma_start(un_t[:], un.ap()).then_inc(s_un, 16)
                for g in range(3, len(GROUPS)):
                    hr_dma(g)
                sync.dma_start(lab_t[:], lab.ap()).then_inc(s_lab, 16)
                # par4 columns: [P (ACT), A (DVE), B (ACT), n (DVE)]
                sync.wait_ge(s_par4, 4)
                sync.dma_start(out4.ap(), par4[:]).then_inc(s_out, 16)

            @block.gpsimd
            def _(gpsimd):
                # NRT's injected postamble does sema_reset between
                # executions, so no explicit sem clearing is needed here.
                gpsimd.memset(zeros[:], 0.0)
                gpsimd.memset(neg40[:], -MASK_BIG).then_inc(s_const, 1)

            @block.scalar
            def _(scalar):
                scalar.wait_ge(s_const, 1)
                waited = -1
                for t in range(ROW_TILES):
                    g = grp_of(t)
                    if g > waited:
                        scalar.wait_ge(s_hr[g], 16)
                        waited = g
                    ins = nc.scalar.activation(
                        sq[:, t * D:(t + 1) * D],
                        xbuf[:, t * D:(t + 1) * D], AF.Square,
                        bias=zeros[:], accum_out=ssq[:, t:t + 1],
                    )
                    if t == ROW_TILES - 1:
                        ins.then_inc(s_sact, 1)
                # same-engine RAW on ssq needs a sem hop (deep pipeline)
                scalar.wait_ge(s_sact, 1)
                if RSQRT_ON_ACT:
                    # rinv = 1/sqrt(ssq) in one ACT op (accuracy validated
                    # against the reference on hardware)
                    nc.scalar.activation(
                        rinv[:], ssq[:], AF.Abs_reciprocal_sqrt,
                        bias=zeros[:]
                    ).then_inc(s_rs, 1)
                else:
                    nc.scalar.activation(
                        rs[:], ssq[:], AF.Sqrt, bias=zeros[:]
                    ).then_inc(s_rs, 1)
                scalar.wait_ge(s_fold, 1)
                nc.scalar.activation(
                    ev[:], simp[:], AF.Exp, bias=zeros[:],
                    accum_out=par4[:, 0:1],
                ).then_inc(s_par4, 1)
                nc.scalar.activation(
                    em[:], simp[:], AF.Exp, bias=neg40[:], scale=-1.0,
                    accum_out=par4[:, 2:3],
                ).then_inc(s_par4, 1)

            @block.vector
            def _(vector):
                vector.wait_ge(s_un, 16)
                waited = -1
                for t in range(ROW_TILES):
                    g = grp_of(t)
                    if g > waited:
                        vector.wait_ge(s_hr[g], 16)
                        waited = g
                    x = xbuf[:, t * D:(t + 1) * D]
                    nc.vector.scalar_tensor_tensor(
                        out=mo[:, t * D:(t + 1) * D], in0=x, scalar=1.0,
                        in1=un_t[:], op0=ALU.mult, op1=ALU.mult,
                        accum_out=dot[:, t:t + 1],
                    ).then_inc(s_sim, 1)
                vector.wait_ge(s_lab, 16)
                nc.vector.tensor_scalar(
                    out=nm[:], in0=lab_t[:], scalar1=float(label_last),
                    scalar2=None, op0=ALU.not_equal,
                ).then_inc(s_mask, 1)
                vector.wait_ge(s_mask, 1)
                nc.vector.tensor_reduce(
                    par4[:, 3:4], nm[:],
                    axis=mybir.AxisListType.X, op=ALU.add,
                ).then_inc(s_par4, 1)
                vector.wait_ge(s_rs, 1)
                if not RSQRT_ON_ACT:
                    nc.vector.reciprocal(rinv[:], rs[:]).then_inc(s_rv, 1)
                    vector.wait_ge(s_rv, 1)  # same-engine RAW: rinv -> sim
                vector.wait_ge(s_sim, ROW_TILES)
                nc.vector.tensor_mul(sim[:], dot[:], rinv[:]).then_inc(
                    s_sim, 1
                )
                vector.wait_ge(s_sim, ROW_TILES + 1)
                nc.vector.scalar_tensor_tensor(
                    out=simp[:], in0=nm[:], scalar=-MASK_BIG, in1=sim[:],
                    op0=ALU.mult, op1=ALU.add,
                ).then_inc(s_fold, 1)
                # A = sum_neg sim
                nc.vector.scalar_tensor_tensor(
                    out=tt1[:], in0=nm[:], scalar=1.0, in1=sim[:],
                    op0=ALU.mult, op1=ALU.mult, accum_out=par4[:, 1:2],
                ).then_inc(s_par4, 1)

    if walrus_fix:
        _split_multiwaits(nc)
    return nc


def _prep_in_maps(h_f, labels_f, h_r, labels_r, bcast_un=True):
    h_f = np.ascontiguousarray(np.asarray(h_f, dtype=np.float32))
    h_r = np.ascontiguousarray(np.asarray(h_r, dtype=np.float32))
    lf = np.asarray(labels_f)
    lr = np.asarray(labels_r)

    u = h_f[-1].astype(np.float32)
    nu = np.maximum(np.sqrt(np.sum(u.astype(np.float32) * u, dtype=np.float32)),
                    np.float32(COS_EPS))
    u_n = np.ascontiguousarray((u / nu).astype(np.float32).reshape(1, D))
    if bcast_un:
        u_n = np.ascontiguousarray(np.broadcast_to(u_n, (128, D)))

    label_last = float(lf[-1])

    in_maps = []
    for c in range(N_CORES):
        rows = slice(c * ROWS_PER_CORE, (c + 1) * ROWS_PER_CORE)
        hr_shard = np.ascontiguousarray(h_r[rows])
        lab_shard = np.ascontiguousarray(
            lr[rows].astype(np.float32).reshape(ROW_TILES, 128).T
        )
        in_maps.append({"hr": hr_shard, "un": u_n, "lab": lab_shard})
    return in_maps, label_last


def _combine(parts):
    """parts: per-core [*,4] partial-sum arrays (raw: [128,4] per-partition
    partials, tile: [1,4]) -> scalar loss (host all-reduce)."""
    agg = np.sum(
        [p.astype(np.float64).reshape(-1, 4).sum(axis=0) for p in parts],
        axis=0,
    )
    S, A, Bsum, n = agg
    lt_sum = A + EPS * S * Bsum - n * np.log(S)
    loss = -lt_sum / (n + 1.0) / B_TOTAL
    return np.array(loss, dtype=np.float32)


TRACE = False          # set by test.py to collect an NTFF profile
LAST_RESULT = None     # BassKernelResults of the most recent run
IMPL = "raw"           # "raw" (hand-scheduled) or "tile"


def kernel(h_f, labels_f, h_r, labels_r, _cache={}):
    global LAST_RESULT
    in_maps, label_last = _prep_in_maps(
        h_f, labels_f, h_r, labels_r, bcast_un=(IMPL == "raw")
    )
    key = (IMPL, label_last)
    if key not in _cache:
        builder = _build_nc_raw if IMPL == "raw" else _build_nc
        _cache[key] = builder(label_last)
    nc = _cache[key]
    res = run_bass_kernel_spmd(
        nc, in_maps, core_ids=list(range(N_CORES)), trace=TRACE
    )
    LAST_RESULT = res
    parts = [res.results[c]["out4"] for c in range(N_CORES)]
    return _combine(parts)

